# revision 41
# baseline (speedup 1.0000x reference)
"""Causal self-attention (B=2, S=2048, E=1024, H=16) on 8 trn2 NeuronCores.

Sharding: data-parallel over batch x tensor-parallel over heads. Core c
computes heads {2c, 2c+1} for both batches (4 (b,h) pairs/core), plus the
row-slice of the output projection for its heads. Host sums the 8 partial
output-projection products and adds bp.

Device layout notes:
  - All projections produce *transposed* activations (qT/kT/vT: [cols, tok])
    so the E-contraction runs on the PE partition dim with fp32r at
    1 cyc/row. x is pre-transposed on the host (xT [E, B*S]).
  - Everything feeding a matmul is typed float32r end-to-end (BIR verifier
    requires matmul operands to be produced as fp32r).
  - qT/kT are zero-padded to K=128 (head data on its own 64-partition half)
    because fp32r matmuls with K=64 run ~1.75x slower than dense K=128.
  - Scores are s.T [ktok, q] tiles; softmax runs without max-subtraction
    (scores bounded ~|8.6| here), the denominator comes from a ones-column
    appended to v, and the divide is a per-(b,h)-batched reciprocal +
    gpsimd partition_broadcast + multiply (engine partition writes must be
    32-aligned, so the denominator rows bounce through a DRAM scratch).
  - Program order is hand-interleaved (engines execute their streams in
    order): batch 0's output projection is emitted inside batch 1's
    attention so the PE never idles waiting on the division chain.
"""

import numpy as np

B, S, E, H = 2, 2048, 1024, 16
HD = E // H  # 64
N_CORES = 8
HPC = H // N_CORES  # heads per core = 2
CW = HPC * HD  # per-core projection width = 128
T = B * S  # 4096 token rows
QB = 512  # q-block width
KT = 128  # k-tile height
NQB = S // QB  # 4 q blocks per (b,h)
NTB = T // QB  # 8 token blocks for projections
NTB_B = NTB // B  # token blocks per batch
NKC = E // 128  # 8 contraction chunks
NVS = S // KT  # 16 v slots per (b,h)

_nc_cache = {}


def _build_kernel():
    import concourse.mybir as mybir
    import concourse.tile as tile
    from concourse import bacc
    from concourse.masks import make_identity

    f32 = mybir.dt.float32
    f32r = mybir.dt.float32r
    bf16 = mybir.dt.bfloat16
    AF = mybir.ActivationFunctionType

    nc = bacc.Bacc("TRN2", target_bir_lowering=False, debug=False)

    xT = nc.dram_tensor("xT", [E, T], f32r, kind="ExternalInput")
    wq = nc.dram_tensor("wq", [E, CW], f32r, kind="ExternalInput")
    wk = nc.dram_tensor("wk", [E, CW], f32r, kind="ExternalInput")
    wv = nc.dram_tensor("wv", [E, CW], f32r, kind="ExternalInput")
    wp = nc.dram_tensor("wp", [CW, E], f32r, kind="ExternalInput")
    bqkv = nc.dram_tensor("bqkv", [3, CW], f32, kind="ExternalInput")
    trimask = nc.dram_tensor("trimask", [128, KT], f32, kind="ExternalInput")
    outT = nc.dram_tensor("outT", [E, T], f32, kind="ExternalOutput")

    xT_r = xT.rearrange("(kc p) n -> p kc n", p=128)
    wq_r = wq.rearrange("(kc p) m -> p kc m", p=128)
    wk_r = wk.rearrange("(kc p) m -> p kc m", p=128)
    wv_r = wv.rearrange("(kc p) m -> p kc m", p=128)

    with tile.TileContext(nc) as tc:
        with (
            tc.tile_pool(name="persist", bufs=1) as persist,
            tc.tile_pool(name="xin", bufs=2) as xin,
            tc.tile_pool(name="ptile", bufs=4) as ptile,
            tc.tile_pool(name="oev", bufs=4) as oev,
            tc.tile_pool(name="small", bufs=2) as small,
            tc.tile_pool(name="dpool", bufs=1) as dpool,
            tc.tile_pool(name="qkv_ps", bufs=3, space="PSUM") as qkv_ps,
            tc.tile_pool(name="s_ps", bufs=3, space="PSUM") as s_ps,
            tc.tile_pool(name="y_ps", bufs=2, space="PSUM") as y_ps,
            tc.tile_pool(name="dscratch", bufs=2, space="DRAM") as dscratch,
        ):
            # ---- resident tiles ----
            # per-(g,kc) weight DMAs, wq-kc0 first: the first projection
            # matmul only needs w_sb[:,0,0] + one x chunk
            w_sb = persist.tile([128, 3, NKC, 128], f32r)  # wq|wk|wv chunks
            w_srcs = (wq_r, wk_r, wv_r)
            for kc in range(NKC):
                for g in range(3):
                    nc.sync.dma_start(
                        out=w_sb[:, g, kc], in_=w_srcs[g][:, kc]
                    )
            b_sb = persist.tile([128, 3], f32)
            nc.sync.dma_start(
                out=b_sb, in_=bqkv.rearrange("three p -> p three")
            )
            mask_sb = persist.tile([128, KT], f32)
            nc.sync.dma_start(out=mask_sb, in_=trimask[:, :])
            wp_sb = persist.tile([128, E], f32r)
            nc.sync.dma_start(out=wp_sb, in_=wp[:, :])

            qT = persist.tile([128, B * HPC * S], f32r)
            kT = persist.tile([128, B * HPC * S], f32r)
            # vT holds one batch at a time (proj writes then transposes
            # read, strictly before the next batch's projections)
            vT = persist.tile([128, S], f32)
            yT = persist.tile([128, T], f32r)
            # v in [ktok, hd+1] layout; ones column at offset 64 of each slot
            v_s = persist.tile([128, B * HPC * NVS, HD + 1], f32r)

            ones_t = persist.tile([128, B * HPC * NVS, 1], f32)
            nc.vector.memset(ones_t, 1.0)
            nc.vector.tensor_copy(out=v_s[:, :, HD : HD + 1], in_=ones_t)
            zeros_t = persist.tile([128, 1], f32)
            nc.vector.memset(zeros_t, 0.0)
            for t in (qT, kT):
                for b in range(B):
                    for hl in range(HPC):
                        zlo, zhi = (HD, 128) if hl == 0 else (0, HD)
                        nc.vector.tensor_copy(
                            out=t[
                                zlo:zhi,
                                (b * HPC + hl) * S : (b * HPC + hl + 1) * S,
                            ],
                            in_=zeros_t[zlo:zhi, 0:1].to_broadcast((HD, S)),
                        )

            yu = persist.tile([128, B * NQB, QB], f32)
            # denominator staging at 32-aligned partition rows
            dcol = dpool.tile([128, 2, QB], f32)

            # identity for PE transpose, per 64-partition half
            ident = persist.tile([128, HD], f32)
            make_identity(nc, ident[0:HD, :])
            make_identity(nc, ident[HD : 2 * HD, :])

            SCALE = 1.0 / float(np.sqrt(HD))
            dests = (qT, kT, vT)

            def transpose_v_tb(b, tbl):
                for hl in range(HPC):
                    for kt in range(tbl * 4, tbl * 4 + 4):
                        slot = (b * HPC + hl) * NVS + kt
                        tp = qkv_ps.tile([KT, HD], f32, tag="ps")
                        nc.tensor.transpose(
                            tp,
                            in_=vT[
                                hl * HD : (hl + 1) * HD,
                                kt * KT : (kt + 1) * KT,
                            ],
                            identity=ident[hl * HD : (hl + 1) * HD, :],
                        )
                        nc.vector.tensor_copy(out=v_s[:, slot, 0:HD], in_=tp)

            def project_batch(b):
                for tbl in range(NTB_B):
                    tb = b * NTB_B + tbl
                    xt = xin.tile([128, NKC, QB], f32r, tag="xt")
                    for kc in range(NKC):  # per-kc DMA: first mm starts early
                        nc.sync.dma_start(
                            out=xt[:, kc],
                            in_=xT_r[:, kc, tb * QB : (tb + 1) * QB],
                        )
                    for g in range(3):
                        ps = qkv_ps.tile([128, QB], f32, tag="ps")
                        for kc in range(NKC):
                            nc.tensor.matmul(
                                ps,
                                lhsT=w_sb[:, g, kc],
                                rhs=xt[:, kc],
                                start=(kc == 0),
                                stop=(kc == NKC - 1),
                            )
                        if g == 2:  # v: packed layout, single eviction
                            nc.scalar.activation(
                                out=vT[:, tbl * QB : (tbl + 1) * QB],
                                in_=ps,
                                func=AF.Identity,
                                bias=b_sb[:, g : g + 1],
                            )
                        else:  # q/k: split per head into padded layout
                            for hl in range(HPC):
                                col = (b * HPC + hl) * S + tbl * QB
                                nc.scalar.activation(
                                    out=dests[g][
                                        hl * HD : (hl + 1) * HD,
                                        col : col + QB,
                                    ],
                                    in_=ps[hl * HD : (hl + 1) * HD, :],
                                    func=AF.Identity,
                                    bias=b_sb[
                                        hl * HD : (hl + 1) * HD, g : g + 1
                                    ],
                                    scale=SCALE if g == 0 else 1.0,
                                )
                    transpose_v_tb(b, tbl)

            def emit_outproj(tb, oc):
                op = qkv_ps.tile([128, QB], f32, tag="ps")
                nc.tensor.matmul(
                    op,
                    lhsT=wp_sb[:, oc * 128 : (oc + 1) * 128],
                    rhs=yT[:, tb * QB : (tb + 1) * QB],
                    start=True,
                    stop=True,
                )
                ot = oev.tile([128, QB], f32)
                if (oc + tb) % 2 == 0:
                    nc.vector.tensor_copy(out=ot, in_=op)
                else:
                    nc.scalar.activation(out=ot, in_=op, func=AF.Copy)
                nc.sync.dma_start(
                    out=outT[
                        oc * 128 : (oc + 1) * 128, tb * QB : (tb + 1) * QB
                    ],
                    in_=ot,
                )

            def attention_block(b, hl, qb):
                qh = qT[:, (b * HPC + hl) * S : (b * HPC + hl + 1) * S]
                kh = kT[:, (b * HPC + hl) * S : (b * HPC + hl + 1) * S]
                nkt = (qb + 1) * (QB // KT)
                yp = y_ps.tile([HD + 1, QB], f32)
                for kt in range(nkt):
                    sp = s_ps.tile([KT, QB], f32)
                    nc.tensor.matmul(
                        sp,
                        lhsT=kh[:, kt * KT : (kt + 1) * KT],
                        rhs=qh[:, qb * QB : (qb + 1) * QB],
                        start=True,
                        stop=True,
                    )
                    pt = ptile.tile([KT, QB], f32r)
                    di = kt - qb * (QB // KT)
                    if di < 0:  # fully-valid tile
                        nc.scalar.activation(out=pt, in_=sp, func=AF.Exp)
                    else:
                        z0 = di * KT
                        if z0 > 0:
                            nc.vector.tensor_copy(
                                out=pt[:, 0:z0],
                                in_=zeros_t[:, 0:1].to_broadcast((KT, z0)),
                            )
                        nc.scalar.activation(
                            out=pt[:, z0:QB], in_=sp[:, z0:QB], func=AF.Exp
                        )
                        nc.vector.tensor_mul(
                            pt[:, z0 : z0 + KT], pt[:, z0 : z0 + KT], mask_sb
                        )
                    slot = (b * HPC + hl) * NVS + kt
                    nc.tensor.matmul(
                        yp,
                        lhsT=v_s[:, slot],
                        rhs=pt,
                        start=(kt == 0),
                        stop=(kt == nkt - 1),
                    )
                nc.vector.tensor_copy(
                    out=yu[hl * HD : (hl + 1) * HD, b * NQB + qb, :],
                    in_=yp[0:HD, :],
                )
                bh = b * HPC + hl
                nc.vector.tensor_copy(
                    out=dcol[32 * bh : 32 * bh + 1, qb % 2, :],
                    in_=yp[HD : HD + 1, :],
                )

            def divide_qb(b, qb):
                """Divide both heads for one q-block, then emit its outproj."""
                gdram = dscratch.tile([2, 1, QB], f32, tag="gd2")
                for hl in range(HPC):
                    bh = b * HPC + hl
                    nc.sync.dma_start(
                        out=gdram[hl : hl + 1],
                        in_=dcol[32 * bh : 32 * bh + 1, qb % 2 : qb % 2 + 1, :],
                    )
                gthb = small.tile([HPC, QB], f32, tag="gth")
                nc.sync.dma_start(
                    out=gthb, in_=gdram.rearrange("p one f -> (p one) f")
                )
                rcpb = small.tile([HPC, QB], f32, tag="gth")
                nc.vector.reciprocal(out=rcpb, in_=gthb)
                for hl in range(HPC):
                    btmp = small.tile([1, QB], f32)
                    nc.sync.dma_start(out=btmp, in_=rcpb[hl : hl + 1, :])
                    rb = small.tile([128, QB], f32)
                    nc.gpsimd.partition_broadcast(out_ap=rb, in_ap=btmp)
                    nc.vector.tensor_mul(
                        yT[
                            hl * HD : (hl + 1) * HD,
                            b * S + qb * QB : b * S + (qb + 1) * QB,
                        ],
                        yu[hl * HD : (hl + 1) * HD, b * NQB + qb, :],
                        rb[hl * HD : (hl + 1) * HD, :],
                    )


            # ================= schedule =================
            project_batch(0)
            for qb in range(NQB):
                for hl in range(HPC):
                    attention_block(0, hl, qb)
                divide_qb(0, qb)

            project_batch(1)
            # batch-0 outproj interleaved into batch-1 attention
            b0_work = [
                (tb, oc) for tb in range(NTB_B) for oc in range(E // 128)
            ]
            prev_qb = None
            for qb in range(NQB):
                for hl in range(HPC):
                    attention_block(1, hl, qb)
                    for _ in range(4):
                        if b0_work:
                            emit_outproj(*b0_work.pop(0))
                divide_qb(1, qb)
                if prev_qb is not None:  # prior division now complete
                    for oc in range(E // 128):
                        emit_outproj(NTB_B + prev_qb, oc)
                prev_qb = qb
            while b0_work:
                emit_outproj(*b0_work.pop(0))
            for oc in range(E // 128):
                emit_outproj(NTB_B + prev_qb, oc)

    nc.compile()
    return nc


def kernel(x, Wq, bq, Wk, bk, Wv, bv, Wp, bp, n_head):
    from concourse.bass_utils import run_bass_kernel_spmd

    x = np.asarray(x, dtype=np.float32)
    Wq, bq = np.asarray(Wq, np.float32), np.asarray(bq, np.float32)
    Wk, bk = np.asarray(Wk, np.float32), np.asarray(bk, np.float32)
    Wv, bv = np.asarray(Wv, np.float32), np.asarray(bv, np.float32)
    Wp, bp = np.asarray(Wp, np.float32), np.asarray(bp, np.float32)
    assert int(n_head) == H and x.shape == (B, S, E)

    if "nc" not in _nc_cache:
        _nc_cache["nc"] = _build_kernel()
    nc = _nc_cache["nc"]

    xT = np.ascontiguousarray(x.reshape(T, E).T)
    scale = 1.0 / np.sqrt(HD)

    # triangle mask for the diagonal block: mask[p, j] = (p <= j)
    p_idx = np.arange(128)[:, None]
    j_idx = np.arange(KT)[None, :]
    mk = (p_idx <= j_idx).astype(np.float32)

    in_maps = []
    for c in range(N_CORES):
        cs = slice(c * CW, (c + 1) * CW)
        bqkv = np.stack([bq[cs] * scale, bk[cs], bv[cs]]).astype(np.float32)
        in_maps.append(
            {
                "xT": xT,
                "wq": np.ascontiguousarray(Wq[:, cs]),
                "wk": np.ascontiguousarray(Wk[:, cs]),
                "wv": np.ascontiguousarray(Wv[:, cs]),
                "wp": np.ascontiguousarray(Wp[cs, :]),
                "bqkv": bqkv,
                "trimask": mk,
            }
        )

    res = run_bass_kernel_spmd(nc, in_maps, list(range(N_CORES)))
    _nc_cache["last_result"] = res

    acc = res.results[0]["outT"].astype(np.float32)
    for c in range(1, N_CORES):
        acc = acc + res.results[c]["outT"]
    out = acc.T + bp[None, :]
    return np.ascontiguousarray(out.reshape(B, S, E).astype(np.float32))


# revision 42
# speedup vs baseline: 1.0471x; 1.0471x over previous
"""Causal self-attention (B=2, S=2048, E=1024, H=16) on 8 trn2 NeuronCores.

Sharding: data-parallel over batch x tensor-parallel over heads. Core c
computes heads {2c, 2c+1} for both batches (4 (b,h) pairs/core), plus the
row-slice of the output projection for its heads. Host sums the 8 partial
output-projection products and adds bp.

Device layout notes:
  - All projections produce *transposed* activations (qT/kT/vT: [cols, tok])
    so the E-contraction runs on the PE partition dim with fp32r at
    1 cyc/row. x is pre-transposed on the host (xT [E, B*S]).
  - Everything feeding a matmul is typed float32r end-to-end (BIR verifier
    requires matmul operands to be produced as fp32r).
  - qT/kT are zero-padded to K=128 (head data on its own 64-partition half)
    because fp32r matmuls with K=64 run ~1.75x slower than dense K=128.
  - Scores are s.T [ktok, q] tiles; softmax runs without max-subtraction
    (scores bounded ~|8.6| here), the denominator comes from a ones-column
    appended to v, and the divide is a per-(b,h)-batched reciprocal +
    gpsimd partition_broadcast + multiply (engine partition writes must be
    32-aligned, so the denominator rows bounce through a DRAM scratch).
  - Program order is hand-interleaved (engines execute their streams in
    order): batch 0's output projection is emitted inside batch 1's
    attention so the PE never idles waiting on the division chain.
"""

import numpy as np

B, S, E, H = 2, 2048, 1024, 16
HD = E // H  # 64
N_CORES = 8
HPC = H // N_CORES  # heads per core = 2
CW = HPC * HD  # per-core projection width = 128
T = B * S  # 4096 token rows
QB = 512  # q-block width
KT = 128  # k-tile height
NQB = S // QB  # 4 q blocks per (b,h)
NTB = T // QB  # 8 token blocks for projections
NTB_B = NTB // B  # token blocks per batch
NKC = E // 128  # 8 contraction chunks
NVS = S // KT  # 16 v slots per (b,h)

_nc_cache = {}


def _build_kernel():
    import concourse.mybir as mybir
    import concourse.tile as tile
    from concourse import bacc
    from concourse.masks import make_identity

    f32 = mybir.dt.float32
    f32r = mybir.dt.float32r
    bf16 = mybir.dt.bfloat16
    AF = mybir.ActivationFunctionType

    nc = bacc.Bacc("TRN2", target_bir_lowering=False, debug=False)

    xT = nc.dram_tensor("xT", [E, T], f32r, kind="ExternalInput")
    wq = nc.dram_tensor("wq", [E, CW], f32r, kind="ExternalInput")
    wk = nc.dram_tensor("wk", [E, CW], f32r, kind="ExternalInput")
    wv = nc.dram_tensor("wv", [E, CW], f32r, kind="ExternalInput")
    wp = nc.dram_tensor("wp", [CW, E], f32r, kind="ExternalInput")
    bqkv = nc.dram_tensor("bqkv", [3, CW], f32, kind="ExternalInput")
    trimask = nc.dram_tensor("trimask", [128, KT], f32, kind="ExternalInput")
    outT = nc.dram_tensor("outT", [E, T], f32, kind="ExternalOutput")

    xT_r = xT.rearrange("(kc p) n -> p kc n", p=128)
    wq_r = wq.rearrange("(kc p) m -> p kc m", p=128)
    wk_r = wk.rearrange("(kc p) m -> p kc m", p=128)
    wv_r = wv.rearrange("(kc p) m -> p kc m", p=128)

    with tile.TileContext(nc) as tc:
        with (
            tc.tile_pool(name="persist", bufs=1) as persist,
            tc.tile_pool(name="xin", bufs=2) as xin,
            tc.tile_pool(name="ptile", bufs=4) as ptile,
            tc.tile_pool(name="oev", bufs=4) as oev,
            tc.tile_pool(name="small", bufs=2) as small,
            tc.tile_pool(name="dpool", bufs=1) as dpool,
            tc.tile_pool(name="qkv_ps", bufs=3, space="PSUM") as qkv_ps,
            tc.tile_pool(name="s_ps", bufs=3, space="PSUM") as s_ps,
            tc.tile_pool(name="y_ps", bufs=2, space="PSUM") as y_ps,
            tc.tile_pool(name="dscratch", bufs=2, space="DRAM") as dscratch,
        ):
            # ---- resident tiles ----
            w_sb = persist.tile([128, 3, NKC, 128], f32r)  # wq|wk|wv chunks
            nc.sync.dma_start(out=w_sb[:, 0], in_=wq_r)
            nc.sync.dma_start(out=w_sb[:, 1], in_=wk_r)
            nc.sync.dma_start(out=w_sb[:, 2], in_=wv_r)
            b_sb = persist.tile([128, 3], f32)
            nc.sync.dma_start(
                out=b_sb, in_=bqkv.rearrange("three p -> p three")
            )
            mask_sb = persist.tile([128, KT], f32)
            nc.sync.dma_start(out=mask_sb, in_=trimask[:, :])
            wp_sb = persist.tile([128, E], f32r)
            nc.sync.dma_start(out=wp_sb, in_=wp[:, :])

            qT = persist.tile([128, B * HPC * S], f32r)
            kT = persist.tile([128, B * HPC * S], f32r)
            # vT holds one batch at a time (proj writes then transposes
            # read, strictly before the next batch's projections)
            vT = persist.tile([128, S], f32)
            yT = persist.tile([128, T], f32r)
            # v in [ktok, hd+1] layout; ones column at offset 64 of each slot
            v_s = persist.tile([128, B * HPC * NVS, HD + 1], f32r)

            ones_t = persist.tile([128, B * HPC * NVS, 1], f32)
            nc.vector.memset(ones_t, 1.0)
            nc.vector.tensor_copy(out=v_s[:, :, HD : HD + 1], in_=ones_t)
            zeros_t = persist.tile([128, 1], f32)
            nc.vector.memset(zeros_t, 0.0)
            for t in (qT, kT):
                for b in range(B):
                    for hl in range(HPC):
                        zlo, zhi = (HD, 128) if hl == 0 else (0, HD)
                        nc.vector.tensor_copy(
                            out=t[
                                zlo:zhi,
                                (b * HPC + hl) * S : (b * HPC + hl + 1) * S,
                            ],
                            in_=zeros_t[zlo:zhi, 0:1].to_broadcast((HD, S)),
                        )

            yu = persist.tile([128, B * NQB, QB], f32)
            # denominator staging at 32-aligned partition rows
            dcol = dpool.tile([128, 2, QB], f32)

            # identity for PE transpose, per 64-partition half
            ident = persist.tile([128, HD], f32)
            make_identity(nc, ident[0:HD, :])
            make_identity(nc, ident[HD : 2 * HD, :])

            SCALE = 1.0 / float(np.sqrt(HD))
            dests = (qT, kT, vT)

            def transpose_v_tb(b, tbl):
                for hl in range(HPC):
                    for kt in range(tbl * 4, tbl * 4 + 4):
                        slot = (b * HPC + hl) * NVS + kt
                        tp = qkv_ps.tile([KT, HD], f32, tag="ps")
                        nc.tensor.transpose(
                            tp,
                            in_=vT[
                                hl * HD : (hl + 1) * HD,
                                kt * KT : (kt + 1) * KT,
                            ],
                            identity=ident[hl * HD : (hl + 1) * HD, :],
                        )
                        nc.vector.tensor_copy(out=v_s[:, slot, 0:HD], in_=tp)

            def project_batch(b):
                for tbl in range(NTB_B):
                    tb = b * NTB_B + tbl
                    xt = xin.tile([128, NKC, QB], f32r, tag="xt")
                    for kh2 in range(2):  # halves; issued off the sync queue
                        nc.gpsimd.dma_start(
                            out=xt[:, kh2 * 4 : kh2 * 4 + 4],
                            in_=xT_r[
                                :,
                                kh2 * 4 : kh2 * 4 + 4,
                                tb * QB : (tb + 1) * QB,
                            ],
                        )
                    for g in range(3):
                        ps = qkv_ps.tile([128, QB], f32, tag="ps")
                        for kc in range(NKC):
                            nc.tensor.matmul(
                                ps,
                                lhsT=w_sb[:, g, kc],
                                rhs=xt[:, kc],
                                start=(kc == 0),
                                stop=(kc == NKC - 1),
                            )
                        if g == 2:  # v: packed layout, single eviction
                            nc.scalar.activation(
                                out=vT[:, tbl * QB : (tbl + 1) * QB],
                                in_=ps,
                                func=AF.Identity,
                                bias=b_sb[:, g : g + 1],
                            )
                        else:  # q/k: split per head into padded layout
                            for hl in range(HPC):
                                col = (b * HPC + hl) * S + tbl * QB
                                nc.scalar.activation(
                                    out=dests[g][
                                        hl * HD : (hl + 1) * HD,
                                        col : col + QB,
                                    ],
                                    in_=ps[hl * HD : (hl + 1) * HD, :],
                                    func=AF.Identity,
                                    bias=b_sb[
                                        hl * HD : (hl + 1) * HD, g : g + 1
                                    ],
                                    scale=SCALE if g == 0 else 1.0,
                                )
                    transpose_v_tb(b, tbl)

            def emit_outproj(tb, oc):
                op = qkv_ps.tile([128, QB], f32, tag="ps")
                nc.tensor.matmul(
                    op,
                    lhsT=wp_sb[:, oc * 128 : (oc + 1) * 128],
                    rhs=yT[:, tb * QB : (tb + 1) * QB],
                    start=True,
                    stop=True,
                )
                ot = oev.tile([128, QB], f32)
                if (oc + tb) % 2 == 0:
                    nc.vector.tensor_copy(out=ot, in_=op)
                else:
                    nc.scalar.activation(out=ot, in_=op, func=AF.Copy)
                nc.sync.dma_start(
                    out=outT[
                        oc * 128 : (oc + 1) * 128, tb * QB : (tb + 1) * QB
                    ],
                    in_=ot,
                )

            def attention_block(b, hl, qb):
                qh = qT[:, (b * HPC + hl) * S : (b * HPC + hl + 1) * S]
                kh = kT[:, (b * HPC + hl) * S : (b * HPC + hl + 1) * S]
                nkt = (qb + 1) * (QB // KT)
                yp = y_ps.tile([HD + 1, QB], f32)
                for kt in range(nkt):
                    sp = s_ps.tile([KT, QB], f32)
                    nc.tensor.matmul(
                        sp,
                        lhsT=kh[:, kt * KT : (kt + 1) * KT],
                        rhs=qh[:, qb * QB : (qb + 1) * QB],
                        start=True,
                        stop=True,
                    )
                    pt = ptile.tile([KT, QB], f32r)
                    di = kt - qb * (QB // KT)
                    if di < 0:  # fully-valid tile
                        nc.scalar.activation(out=pt, in_=sp, func=AF.Exp)
                    else:
                        z0 = di * KT
                        if z0 > 0:
                            nc.vector.tensor_copy(
                                out=pt[:, 0:z0],
                                in_=zeros_t[:, 0:1].to_broadcast((KT, z0)),
                            )
                        nc.scalar.activation(
                            out=pt[:, z0:QB], in_=sp[:, z0:QB], func=AF.Exp
                        )
                        nc.vector.tensor_mul(
                            pt[:, z0 : z0 + KT], pt[:, z0 : z0 + KT], mask_sb
                        )
                    slot = (b * HPC + hl) * NVS + kt
                    nc.tensor.matmul(
                        yp,
                        lhsT=v_s[:, slot],
                        rhs=pt,
                        start=(kt == 0),
                        stop=(kt == nkt - 1),
                    )
                nc.vector.tensor_copy(
                    out=yu[hl * HD : (hl + 1) * HD, b * NQB + qb, :],
                    in_=yp[0:HD, :],
                )
                bh = b * HPC + hl
                nc.vector.tensor_copy(
                    out=dcol[32 * bh : 32 * bh + 1, qb % 2, :],
                    in_=yp[HD : HD + 1, :],
                )

            def divide_qb(b, qb):
                """Divide both heads for one q-block, then emit its outproj."""
                gdram = dscratch.tile([2, 1, QB], f32, tag="gd2")
                for hl in range(HPC):
                    bh = b * HPC + hl
                    nc.sync.dma_start(
                        out=gdram[hl : hl + 1],
                        in_=dcol[32 * bh : 32 * bh + 1, qb % 2 : qb % 2 + 1, :],
                    )
                gthb = small.tile([HPC, QB], f32, tag="gth")
                nc.sync.dma_start(
                    out=gthb, in_=gdram.rearrange("p one f -> (p one) f")
                )
                rcpb = small.tile([HPC, QB], f32, tag="gth")
                nc.vector.reciprocal(out=rcpb, in_=gthb)
                for hl in range(HPC):
                    btmp = small.tile([1, QB], f32)
                    nc.sync.dma_start(out=btmp, in_=rcpb[hl : hl + 1, :])
                    rb = small.tile([128, QB], f32)
                    nc.gpsimd.partition_broadcast(out_ap=rb, in_ap=btmp)
                    nc.vector.tensor_mul(
                        yT[
                            hl * HD : (hl + 1) * HD,
                            b * S + qb * QB : b * S + (qb + 1) * QB,
                        ],
                        yu[hl * HD : (hl + 1) * HD, b * NQB + qb, :],
                        rb[hl * HD : (hl + 1) * HD, :],
                    )


            # ================= schedule =================
            project_batch(0)
            for qb in range(NQB):
                for hl in range(HPC):
                    attention_block(0, hl, qb)
                divide_qb(0, qb)

            project_batch(1)
            # batch-0 outproj interleaved into batch-1 attention
            b0_work = [
                (tb, oc) for tb in range(NTB_B) for oc in range(E // 128)
            ]
            prev_qb = None
            for qb in range(NQB):
                for hl in range(HPC):
                    attention_block(1, hl, qb)
                    for _ in range(4):
                        if b0_work:
                            emit_outproj(*b0_work.pop(0))
                divide_qb(1, qb)
                if prev_qb is not None:  # prior division now complete
                    for oc in range(E // 128):
                        emit_outproj(NTB_B + prev_qb, oc)
                prev_qb = qb
            while b0_work:
                emit_outproj(*b0_work.pop(0))
            for oc in range(E // 128):
                emit_outproj(NTB_B + prev_qb, oc)

    nc.compile()
    return nc


def kernel(x, Wq, bq, Wk, bk, Wv, bv, Wp, bp, n_head):
    from concourse.bass_utils import run_bass_kernel_spmd

    x = np.asarray(x, dtype=np.float32)
    Wq, bq = np.asarray(Wq, np.float32), np.asarray(bq, np.float32)
    Wk, bk = np.asarray(Wk, np.float32), np.asarray(bk, np.float32)
    Wv, bv = np.asarray(Wv, np.float32), np.asarray(bv, np.float32)
    Wp, bp = np.asarray(Wp, np.float32), np.asarray(bp, np.float32)
    assert int(n_head) == H and x.shape == (B, S, E)

    if "nc" not in _nc_cache:
        _nc_cache["nc"] = _build_kernel()
    nc = _nc_cache["nc"]

    xT = np.ascontiguousarray(x.reshape(T, E).T)
    scale = 1.0 / np.sqrt(HD)

    # triangle mask for the diagonal block: mask[p, j] = (p <= j)
    p_idx = np.arange(128)[:, None]
    j_idx = np.arange(KT)[None, :]
    mk = (p_idx <= j_idx).astype(np.float32)

    in_maps = []
    for c in range(N_CORES):
        cs = slice(c * CW, (c + 1) * CW)
        bqkv = np.stack([bq[cs] * scale, bk[cs], bv[cs]]).astype(np.float32)
        in_maps.append(
            {
                "xT": xT,
                "wq": np.ascontiguousarray(Wq[:, cs]),
                "wk": np.ascontiguousarray(Wk[:, cs]),
                "wv": np.ascontiguousarray(Wv[:, cs]),
                "wp": np.ascontiguousarray(Wp[cs, :]),
                "bqkv": bqkv,
                "trimask": mk,
            }
        )

    res = run_bass_kernel_spmd(nc, in_maps, list(range(N_CORES)))
    _nc_cache["last_result"] = res

    acc = res.results[0]["outT"].astype(np.float32)
    for c in range(1, N_CORES):
        acc = acc + res.results[c]["outT"]
    out = acc.T + bp[None, :]
    return np.ascontiguousarray(out.reshape(B, S, E).astype(np.float32))
